# revision 10
# baseline (speedup 1.0000x reference)
"""Trainium2 Bass kernel for nn_Conditioner (retrieval KNN + gather + projection).

Computation (see reference):
  1. sonata:  nearest = argmin_m ||obj_pts[0][n] - pc[m]||      [8192] (row-sharded across 8 cores)
  2. per part b (= core): idx_g/idx_l = argmin over obj_pts[b] for global/local query tokens
  3. gather feats[nearest[idx]] and project concat([cond, feats]) @ W + b  per branch

Distance argmin as argmax of score = 2*a.b - ||b||^2 (same ordering; row-constant
||a||^2 dropped).  Scores are computed on the PE with a K=24 bf16 triple-split
matmul (a = a0+a1+a2, b = b0+b1+b2 in bf16; the 6 dominant cross terms give
~fp32 accuracy at 1 cycle/column instead of fp32's 4).

Per 128-row chunk the 8192 scores stream through PSUM in two 4096-wide halves:
  - Act copies each 1024-wide PSUM quarter into an SBUF half-row buffer
  - DVE max8 finds each half's max; an elementwise max merges them into the
    global row max V
  - DVE max_index scans each half for V -> first index (exact numpy argmin
    tie semantics via min(global positions), not-found yields sentinel 2^32-1)
(The V3 Pool engine only legally runs iota/collectives/indirect DMA, so the
reduce stays on DVE; the 4x-faster bf16 score matmul and the deep per-chunk
pipeline are where the time goes.)

Sharding: data-parallel over B; sonata row-sharded + AllGather of the 8192
nearest indices; feats/W replicated.  All inputs are laid out on host
(transpose/pack/split to bf16); outputs are fp32.
"""

import os
import sys
import numpy as np

sys.path.insert(0, "/opt/trn_rl_repo")

import concourse.bass as bass
import concourse.bacc as bacc
import concourse.mybir as mybir
import concourse.tile as tile
from concourse import masks
from concourse.bass import IndirectOffsetOnAxis
from concourse.bass_utils import run_bass_kernel_spmd

F32 = mybir.dt.float32
I32 = mybir.dt.int32
U32 = mybir.dt.uint32
BF16 = mybir.dt.bfloat16
Alu = mybir.AluOpType
Axis = mybir.AxisListType

P = 128
B = 8
NOBJ = 8192
M = 8192
NTOK = 1024
DLAT = 1024
DFEAT = 512
DOUT = 1024
NCORES = 8
KS = 32                            # split-3 bf16 score matmul contraction

SON_ROWS = NOBJ // NCORES          # 1024 sonata rows per core
NCH_SON = SON_ROWS // P            # 8 chunks
NCH_GL = 2 * NTOK // P             # 16 chunks (global tokens then local tokens)
NCH = NCH_SON + NCH_GL             # 24
HW_ = 4096                         # score half width (SBUF buffer)
QW = 1024                          # PSUM quarter width

LOOP = int(os.environ.get("KLOOP", "1"))


def build_nc():
    nc = bacc.Bacc(None, target_bir_lowering=False, num_devices=NCORES)

    aT_son = nc.dram_tensor("aT_son", [KS, SON_ROWS], BF16, kind="ExternalInput")
    bT_son = nc.dram_tensor("bT_son", [KS, M], BF16, kind="ExternalInput")
    aT_gl = nc.dram_tensor("aT_gl", [KS, 2 * NTOK], BF16, kind="ExternalInput")
    bT_gl = nc.dram_tensor("bT_gl", [KS, NOBJ], BF16, kind="ExternalInput")
    # condP[br, p, jc, kc, t]: token jc*128+t, latent kc*128+p
    condP = nc.dram_tensor("condP", [2, P, NTOK // P, DLAT // P, P], BF16,
                           kind="ExternalInput")
    # Wp[br, p, kc, d]: W[kc*128+p, d]
    Wp = nc.dram_tensor("Wp", [2, P, (DLAT + DFEAT) // P, DOUT], BF16,
                        kind="ExternalInput")
    bias2 = nc.dram_tensor("bias2", [2, DOUT], BF16, kind="ExternalInput")
    feats = nc.dram_tensor("feats", [M, DFEAT], BF16, kind="ExternalInput")
    obj_out = nc.dram_tensor("obj_out", [NTOK, DOUT], F32, kind="ExternalOutput")
    geo_out = nc.dram_tensor("geo_out", [NTOK, DOUT], F32, kind="ExternalOutput")

    with tile.TileContext(nc) as tc:
        _body(nc, tc, aT_son, bT_son, aT_gl, bT_gl, condP, Wp, bias2, feats,
              obj_out, geo_out)
    nc.compile()
    return nc


def _body(nc, tc, aT_son, bT_son, aT_gl, bT_gl, condP, Wp, bias2, feats,
          obj_out, geo_out):
    from contextlib import ExitStack

    ctx = ExitStack()
    with ctx:
        const = ctx.enter_context(tc.tile_pool(name="const", bufs=1))

        ident_bf = const.tile([P, P], BF16)
        masks.make_identity(nc, ident_bf[:])
        ones_bf = const.tile([1, P], BF16)
        nc.vector.memset(ones_bf[:], 1.0)
        bias_sb = const.tile([1, 2 * DOUT], BF16)
        nc.sync.dma_start(bias_sb[:], bias2[:].rearrange("a b -> (a b)"))
        # hb[p, :] = [0, 4096]: global position base of each score half
        hb = const.tile([P, 2], F32)
        nc.gpsimd.iota(hb[:], pattern=[[HW_, 2]], base=0, channel_multiplier=0,
                       allow_small_or_imprecise_dtypes=True)

        for _it in range(LOOP):
            _iteration(nc, tc, ctx, const, ident_bf, ones_bf, bias_sb, hb,
                       aT_son, bT_son, aT_gl, bT_gl, condP, Wp, feats,
                       obj_out, geo_out)


def _iteration(nc, tc, ctx0, const, ident_bf, ones_bf, bias_sb, hb,
               aT_son, bT_son, aT_gl, bT_gl, condP, Wp, feats,
               obj_out, geo_out):
    from contextlib import ExitStack

    ctx = ExitStack()
    with ctx:
        coord = ctx.enter_context(tc.tile_pool(name="coord", bufs=1))
        wpool = ctx.enter_context(tc.tile_pool(name="wpool", bufs=1))
        scores = ctx.enter_context(tc.tile_pool(name="scores", bufs=4))
        vm8 = ctx.enter_context(tc.tile_pool(name="vm8", bufs=4))
        small = ctx.enter_context(tc.tile_pool(name="small", bufs=1))
        idxp = ctx.enter_context(tc.tile_pool(name="idxp", bufs=4))
        compp = ctx.enter_context(tc.tile_pool(name="compp", bufs=4))
        pfp = ctx.enter_context(tc.tile_pool(name="pfp", bufs=4))
        ftp = ctx.enter_context(tc.tile_pool(name="ftp", bufs=4))
        condpool = ctx.enter_context(tc.tile_pool(name="condpool", bufs=4))
        outp = ctx.enter_context(tc.tile_pool(name="outp", bufs=4))
        dram = ctx.enter_context(tc.tile_pool(name="dram", bufs=1, space="DRAM"))
        psc = ctx.enter_context(tc.tile_pool(name="psc", bufs=2, space="PSUM"))
        pstr = ctx.enter_context(tc.tile_pool(name="pstr", bufs=2, space="PSUM"))
        pspj = ctx.enter_context(tc.tile_pool(name="pspj", bufs=2, space="PSUM"))

        # ---- coordinate tiles (bf16 triple-split, K=24 rows) ----
        cs = coord.tile([KS, M + SON_ROWS], BF16, name="cs", tag="cs")
        nc.sync.dma_start(cs[:, 0:M], bT_son[:])
        nc.sync.dma_start(cs[:, M:M + SON_ROWS], aT_son[:])
        cg = coord.tile([KS, NOBJ + 2 * NTOK], BF16, name="cg", tag="cg")
        nc.sync.dma_start(cg[:, 0:NOBJ], bT_gl[:])
        nc.sync.dma_start(cg[:, NOBJ:NOBJ + 2 * NTOK], aT_gl[:])

        # ---- weights resident (streamed early, used mid-kernel) ----
        wsb = []
        for br in range(2):
            t = wpool.tile([P, (DLAT + DFEAT) // P, DOUT], BF16,
                           name=f"w{br}", tag=f"w{br}")
            nc.sync.dma_start(t[:], Wp[br])
            wsb.append(t)

        # ---- small result buffers ----
        fidx = small.tile([P, NCH, 2], F32)     # per-(chunk,half) global idx
        fmin_son = small.tile([P, NCH_SON], F32)
        idx_son_i = small.tile([P, NCH_SON], I32)
        idx8 = small.tile([P, NCH * 2 * 8], U32)  # raw max_index outputs

        near_in = dram.tile([SON_ROWS, 1], I32)
        near_all = dram.tile([NOBJ, 1], I32)

        # per-token-chunk state created during the pipelined loop
        pf_t = {}      # chunk -> gathered feats tile [P, 512]
        cj_t = {}      # chunk -> cond tile [P, 8, 128]

        def emit_gather(c, idxc):
            """Right after chunk c's argmin: compose with sonata + gather
            feats rows + kick off this token-chunk's cond DMA."""
            b, j = divmod(c - NCH_SON, 8)
            compj = compp.tile([P, 1], I32, name="compj", tag="compj")
            nc.gpsimd.indirect_dma_start(
                out=compj[:], out_offset=None, in_=near_all[:],
                in_offset=IndirectOffsetOnAxis(ap=idxc[:], axis=0))
            pfj = pfp.tile([P, DFEAT], BF16, name="pfj", tag="pfj")
            nc.gpsimd.indirect_dma_start(
                out=pfj[:], out_offset=None, in_=feats[:],
                in_offset=IndirectOffsetOnAxis(ap=compj[:], axis=0))
            pf_t[c] = pfj
            cj = condpool.tile([P, DLAT // P, P], BF16, name="cj", tag="cj")
            nc.sync.dma_start(cj[:], condP[b, :, j])
            cj_t[c] = cj

        def emit_proj(c):
            """Two chunks later: transpose gathered feats and project."""
            b, j = divmod(c - NCH_SON, 8)
            pfj = pf_t.pop(c)
            cj = cj_t.pop(c)
            ftj = ftp.tile([P, DFEAT // P, P], BF16, name="ftj", tag="ftj")
            for fc in range(DFEAT // P):
                pt = pstr.tile([P, P], BF16, name="pt", tag="pt")
                nc.tensor.transpose(pt[:], pfj[:, fc * P:(fc + 1) * P],
                                    ident_bf[:])
                nc.scalar.copy(ftj[:, fc, :], pt[:])
            out_dram = obj_out if b == 0 else geo_out
            for dh in range(DOUT // 512):
                pt = pspj.tile([P, 512], F32, name="ppj", tag="ppj")
                nc.tensor.matmul(
                    pt[:], ones_bf[:],
                    bias_sb[:, b * DOUT + dh * 512: b * DOUT + (dh + 1) * 512],
                    start=True, stop=False)
                for kc in range(DLAT // P):
                    nc.tensor.matmul(
                        pt[:], cj[:, kc, :],
                        wsb[b][:, kc, dh * 512:(dh + 1) * 512],
                        start=False, stop=False)
                nf = DFEAT // P
                for fc in range(nf):
                    nc.tensor.matmul(
                        pt[:], ftj[:, fc, :],
                        wsb[b][:, DLAT // P + fc, dh * 512:(dh + 1) * 512],
                        start=False, stop=(fc == nf - 1))
                ot = outp.tile([P, 512], F32, name="ot", tag="ot")
                nc.scalar.copy(ot[:], pt[:])
                nc.sync.dma_start(
                    out_dram[j * P:(j + 1) * P, dh * 512:(dh + 1) * 512],
                    ot[:])

        # ================= pipelined main loop =================
        # chunk list: (coords tile, query col offset)
        chunks = [(cs, M + j * P) for j in range(NCH_SON)] + \
                 [(cg, NOBJ + j * P) for j in range(NCH_GL)]

        for c, (ct, a_off) in enumerate(chunks):
            sbh = [None, None]
            vmh = [None, None]
            for h in range(2):
                sb = scores.tile([P, HW_], F32, name="sb", tag="sb")
                sbh[h] = sb
                for q in range(HW_ // QW):
                    ps = psc.tile([P, QW], F32, name="ps", tag="ps")
                    for i in range(QW // 512):
                        col = h * HW_ + q * QW + i * 512
                        nc.tensor.matmul(
                            ps[:, i * 512:(i + 1) * 512],
                            ct[:, a_off:a_off + P],
                            ct[:, col:col + 512],
                            start=True, stop=True)
                    nc.scalar.copy(sb[:, q * QW:(q + 1) * QW], ps[:])
                # DVE: top-8 of the half (element 0 = half max)
                vm = vm8.tile([P, 8], F32, name="vm", tag="vm")
                vmh[h] = vm
                nc.vector.max(vm[:], sb[:])
            # global row max V (element 0); other elements are junk keys
            nc.vector.tensor_tensor(out=vmh[0][:], in0=vmh[0][:],
                                    in1=vmh[1][:], op=Alu.max)
            for h in range(2):
                o = (c * 2 + h) * 8
                nc.vector.max_index(idx8[:, o:o + 8], vmh[0][:], sbh[h][:])
            # combine halves: global first index = min(h*4096 + idx_h)
            # (not-found -> 2^32-1, exact in f32 ordering)
            iv = idx8[:, c * 16:(c + 1) * 16].rearrange("p (h e) -> p h e", e=8)
            nc.vector.tensor_copy(fidx[:, c, :], iv[:, :, 0])
            nc.vector.tensor_tensor(out=fidx[:, c, :], in0=fidx[:, c, :],
                                    in1=hb[:], op=Alu.add)
            if c < NCH_SON:
                nc.vector.tensor_tensor(out=fmin_son[:, c:c + 1],
                                        in0=fidx[:, c, 0:1],
                                        in1=fidx[:, c, 1:2], op=Alu.min)
                if c == NCH_SON - 1:
                    nc.vector.tensor_copy(idx_son_i[:], fmin_son[:])
                    nc.sync.dma_start(
                        near_in[:].rearrange("(j p) o -> p (j o)", p=P),
                        idx_son_i[:])
                    nc.gpsimd.collective_compute(
                        "AllGather", Alu.bypass,
                        replica_groups=[list(range(NCORES))],
                        ins=[near_in[:]], outs=[near_all[:]])
            else:
                idxf = idxp.tile([P, 1], F32, name="idxf", tag="idxf")
                nc.vector.tensor_tensor(out=idxf[:], in0=fidx[:, c, 0:1],
                                        in1=fidx[:, c, 1:2], op=Alu.min)
                idxc = idxp.tile([P, 1], I32, name="idxc", tag="idxc")
                nc.vector.tensor_copy(idxc[:], idxf[:])
                emit_gather(c, idxc)
                if c - 2 >= NCH_SON:
                    emit_proj(c - 2)

        emit_proj(NCH - 2)
        emit_proj(NCH - 1)


_NC_CACHE = None


def _get_nc():
    global _NC_CACHE
    if _NC_CACHE is None:
        _NC_CACHE = build_nc()
    return _NC_CACHE


def _split3(x):
    """fp32 -> three bf16 planes whose sum carries ~24 mantissa bits."""
    import ml_dtypes
    BF = ml_dtypes.bfloat16
    x = np.asarray(x, np.float32)
    b0 = x.astype(BF)
    r = x - b0.astype(np.float32)
    b1 = r.astype(BF)
    r2 = r - b1.astype(np.float32)
    b2 = r2.astype(BF)
    return b0, b1, b2


def _pack_queries(q):
    # [n, 3] -> [32, n] rows: split3 of (2x, 2y, 2z, 1) ordered
    # [q0,q0,q1,q0,q1,q2,q1,q2] pairing cands [c0,c1,c0,c2,c1,c0,c2,c1]
    q = np.asarray(q, np.float32)
    aug = np.concatenate([2.0 * q.T, np.ones((1, q.shape[0]), np.float32)], axis=0)
    q0, q1, q2 = _split3(aug)
    return np.ascontiguousarray(
        np.concatenate([q0, q0, q1, q0, q1, q2, q1, q2], axis=0))


def _pack_cands(p):
    # [m, 3] -> [32, m] rows: split3 of (x, y, z, -||p||^2)
    p = np.asarray(p, np.float32)
    aug = np.concatenate([p.T, -np.sum(p * p, axis=1)[None, :]], axis=0)
    c0, c1, c2 = _split3(aug)
    return np.ascontiguousarray(
        np.concatenate([c0, c1, c0, c2, c1, c0, c2, c1], axis=0))


def make_in_maps(object_surface, precomputed_points, precomputed_feats,
                 geo_cond, obj_cond, local_pc_infos, global_pc_infos,
                 W_obj, b_obj, W_geo, b_geo):
    import ml_dtypes
    BF = ml_dtypes.bfloat16

    obj_pts = np.asarray(object_surface, np.float32)[..., :3]
    pts0 = obj_pts[0]
    bT_son = _pack_cands(np.asarray(precomputed_points, np.float32))
    aT_gl = np.ascontiguousarray(np.concatenate(
        [_pack_queries(np.asarray(global_pc_infos, np.float32)[0]),
         _pack_queries(np.asarray(local_pc_infos, np.float32)[0])], axis=1))

    # Wp[br, p, kc, d] = W[kc*128+p, d]
    Ws = np.stack([W_obj, W_geo]).astype(np.float32)
    Wp = np.ascontiguousarray(
        Ws.reshape(2, (DLAT + DFEAT) // P, P, DOUT).transpose(0, 2, 1, 3)
    ).astype(BF)
    bias2 = np.ascontiguousarray(np.stack([b_obj, b_geo]).astype(BF))
    feats = np.ascontiguousarray(np.asarray(precomputed_feats).astype(BF))

    in_maps = []
    for c in range(NCORES):
        # condP[br, p, jc, kc, t] = cond[br][jc*128+t, kc*128+p]
        conds = np.stack([np.asarray(obj_cond)[c], np.asarray(geo_cond)[c]])
        condP = np.ascontiguousarray(
            conds.reshape(2, NTOK // P, P, DLAT // P, P).transpose(0, 4, 1, 3, 2)
        ).astype(BF)
        in_maps.append({
            "aT_son": _pack_queries(pts0[c * SON_ROWS:(c + 1) * SON_ROWS]),
            "bT_son": bT_son,
            "aT_gl": aT_gl,
            "bT_gl": _pack_cands(obj_pts[c]),
            "condP": condP,
            "Wp": Wp,
            "bias2": bias2,
            "feats": feats,
        })
    return in_maps


def kernel(**inputs):
    nc = _get_nc()
    in_maps = make_in_maps(**inputs)
    res = run_bass_kernel_spmd(nc, in_maps, core_ids=list(range(NCORES)))
    obj = np.stack([np.asarray(res.results[c]["obj_out"]).astype(np.float32)
                    for c in range(NCORES)])
    geo = np.stack([np.asarray(res.results[c]["geo_out"]).astype(np.float32)
                    for c in range(NCORES)])
    return obj, geo


if __name__ == "__main__":
    nc = _get_nc()
    print("built + compiled ok")


# revision 23
# speedup vs baseline: 1.4622x; 1.4622x over previous
"""Trainium2 Bass kernel for nn_Conditioner (retrieval KNN + gather + projection).

Computation (see reference):
  1. sonata:  nearest = argmin_m ||obj_pts[0][n] - pc[m]||      [8192] (row-sharded across 8 cores)
  2. per part b (= core): idx_g/idx_l = argmin over obj_pts[b] for global/local query tokens
  3. gather feats[nearest[idx]] and project concat([cond, feats]) @ W + b  per branch

Distance argmin as argmax of score = 2*a.b - ||b||^2 (same ordering; row-constant
||a||^2 dropped).  Scores are computed on the PE with a K=24 bf16 triple-split
matmul (a = a0+a1+a2, b = b0+b1+b2 in bf16; the 6 dominant cross terms give
~fp32 accuracy at 1 cycle/column instead of fp32's 4).

Per 128-row chunk the row max V is computed OFF the DVE via a transposed
score stream running two chunks ahead: PE emits [cand,query] blocks, Act
copies them to SBUF, Pool's partition-direction tensor_reduce collapses the
128 candidates per block into one [1,8192] row, and a DRAM round-trip
rescatters it to [64,128] for a final partition-reduce + a [1,128]->[128,1]
transpose-load giving V per query.  bf16 products are exact and the PE
accumulates along K in the same order for both layouts, so V is bit-identical
to the forward scores: DVE max_index scans the 16 forward PSUM stripes
directly (no forward SBUF copies), not-found stripes yield sentinel 2^32-1,
and min(q*512 + idx_q) restores exact numpy first-index tie semantics.
This halves DVE work; PE (two score streams + projections) is the bottleneck
at ~77% occupancy.

Sharding: data-parallel over B; sonata row-sharded + AllGather of the 8192
nearest indices; feats/W replicated.  All inputs are laid out on host
(transpose/pack/split to bf16); outputs are fp32.
"""

import os
import sys
import numpy as np

sys.path.insert(0, "/opt/trn_rl_repo")

import concourse.bass as bass
import concourse.bacc as bacc
import concourse.mybir as mybir
import concourse.tile as tile
from concourse import masks
from concourse.bass import IndirectOffsetOnAxis
from concourse.bass_utils import run_bass_kernel_spmd

F32 = mybir.dt.float32
I32 = mybir.dt.int32
U32 = mybir.dt.uint32
BF16 = mybir.dt.bfloat16
Alu = mybir.AluOpType
Axis = mybir.AxisListType

P = 128
B = 8
NOBJ = 8192
M = 8192
NTOK = 1024
DLAT = 1024
DFEAT = 512
DOUT = 1024
NCORES = 8
KS = 32                            # split-3 bf16 score matmul contraction

SON_ROWS = NOBJ // NCORES          # 1024 sonata rows per core
NCH_SON = SON_ROWS // P            # 8 chunks
NCH_GL = 2 * NTOK // P             # 16 chunks (global tokens then local tokens)
NCH = NCH_SON + NCH_GL             # 24
HW_ = 4096                         # score half width (SBUF buffer)
QW = 1024                          # PSUM quarter width

LOOP = int(os.environ.get("KLOOP", "1"))


def build_nc():
    nc = bacc.Bacc(None, target_bir_lowering=False, num_devices=NCORES)

    aT_son = nc.dram_tensor("aT_son", [KS, SON_ROWS], BF16, kind="ExternalInput")
    bT_son = nc.dram_tensor("bT_son", [KS, M], BF16, kind="ExternalInput")
    aT_gl = nc.dram_tensor("aT_gl", [KS, 2 * NTOK], BF16, kind="ExternalInput")
    bT_gl = nc.dram_tensor("bT_gl", [KS, NOBJ], BF16, kind="ExternalInput")
    # condP[br, p, jc, kc, t]: token jc*128+t, latent kc*128+p
    condP = nc.dram_tensor("condP", [2, P, NTOK // P, DLAT // P, P], BF16,
                           kind="ExternalInput")
    # Wp[br, p, kc, d]: W[kc*128+p, d]
    Wp = nc.dram_tensor("Wp", [2, P, (DLAT + DFEAT) // P, DOUT], BF16,
                        kind="ExternalInput")
    bias2 = nc.dram_tensor("bias2", [2, DOUT], BF16, kind="ExternalInput")
    feats = nc.dram_tensor("feats", [M, DFEAT], BF16, kind="ExternalInput")
    obj_out = nc.dram_tensor("obj_out", [NTOK, DOUT], F32, kind="ExternalOutput")
    geo_out = nc.dram_tensor("geo_out", [NTOK, DOUT], F32, kind="ExternalOutput")

    with tile.TileContext(nc) as tc:
        _body(nc, tc, aT_son, bT_son, aT_gl, bT_gl, condP, Wp, bias2, feats,
              obj_out, geo_out)
    nc.compile()
    return nc


def _body(nc, tc, aT_son, bT_son, aT_gl, bT_gl, condP, Wp, bias2, feats,
          obj_out, geo_out):
    from contextlib import ExitStack

    ctx = ExitStack()
    with ctx:
        const = ctx.enter_context(tc.tile_pool(name="const", bufs=1))

        ident_bf = const.tile([P, P], BF16)
        masks.make_identity(nc, ident_bf[:])
        ones_bf = const.tile([1, P], BF16)
        nc.vector.memset(ones_bf[:], 1.0)
        bias_sb = const.tile([1, 2 * DOUT], BF16)
        nc.sync.dma_start(bias_sb[:], bias2[:].rearrange("a b -> (a b)"))
        # qb16[p, :] = [0, 512, ..., 7680]: global base of each score stripe
        qb16 = const.tile([P, 16], F32)
        nc.gpsimd.iota(qb16[:], pattern=[[512, 16]], base=0,
                       channel_multiplier=0,
                       allow_small_or_imprecise_dtypes=True)
        ones8 = const.tile([P, 8], F32)
        nc.vector.memset(ones8[:], 1.0)

        for _it in range(LOOP):
            _iteration(nc, tc, ctx, const, ident_bf, ones_bf, bias_sb,
                       qb16, ones8,
                       aT_son, bT_son, aT_gl, bT_gl, condP, Wp, feats,
                       obj_out, geo_out)


def _iteration(nc, tc, ctx0, const, ident_bf, ones_bf, bias_sb,
               qb16, ones8,
               aT_son, bT_son, aT_gl, bT_gl, condP, Wp, feats,
               obj_out, geo_out):
    from contextlib import ExitStack

    ctx = ExitStack()
    with ctx:
        coord = ctx.enter_context(tc.tile_pool(name="coord", bufs=1))
        wpool = ctx.enter_context(tc.tile_pool(name="wpool", bufs=1))
        stgp = ctx.enter_context(tc.tile_pool(name="stgp", bufs=3))
        arp = ctx.enter_context(tc.tile_pool(name="arp", bufs=2))
        f2p = ctx.enter_context(tc.tile_pool(name="f2p", bufs=2))
        vm8 = ctx.enter_context(tc.tile_pool(name="vm8", bufs=4))
        small = ctx.enter_context(tc.tile_pool(name="small", bufs=1))
        idxp = ctx.enter_context(tc.tile_pool(name="idxp", bufs=4))
        compp = ctx.enter_context(tc.tile_pool(name="compp", bufs=4))
        pfp = ctx.enter_context(tc.tile_pool(name="pfp", bufs=4))
        ftp = ctx.enter_context(tc.tile_pool(name="ftp", bufs=4))
        condpool = ctx.enter_context(tc.tile_pool(name="condpool", bufs=4))
        outp = ctx.enter_context(tc.tile_pool(name="outp", bufs=4))
        dram = ctx.enter_context(tc.tile_pool(name="dram", bufs=1, space="DRAM"))
        psc = ctx.enter_context(tc.tile_pool(name="psc", bufs=2, space="PSUM"))
        pst = ctx.enter_context(tc.tile_pool(name="pst", bufs=2, space="PSUM"))
        pstr = ctx.enter_context(tc.tile_pool(name="pstr", bufs=2, space="PSUM"))
        pspj = ctx.enter_context(tc.tile_pool(name="pspj", bufs=2, space="PSUM"))

        # ---- coordinate tiles (bf16 triple-split, K=24 rows) ----
        cs = coord.tile([KS, M + SON_ROWS], BF16, name="cs", tag="cs")
        nc.sync.dma_start(cs[:, M:M + SON_ROWS], aT_son[:])
        nc.sync.dma_start(cs[:, 0:M // 2], bT_son[:, 0:M // 2])
        nc.sync.dma_start(cs[:, M // 2:M], bT_son[:, M // 2:M])
        cg = coord.tile([KS, NOBJ + 2 * NTOK], BF16, name="cg", tag="cg")
        nc.sync.dma_start(cg[:, 0:NOBJ], bT_gl[:])
        nc.sync.dma_start(cg[:, NOBJ:NOBJ + 2 * NTOK], aT_gl[:])

        # ---- weights resident (streamed early, used mid-kernel) ----
        wsb = []
        for br in range(2):
            t = wpool.tile([P, (DLAT + DFEAT) // P, DOUT], BF16,
                           name=f"w{br}", tag=f"w{br}")
            nc.sync.dma_start(t[:], Wp[br])
            wsb.append(t)

        # ---- small result buffers ----
        fmin_son = small.tile([P, NCH_SON], F32)
        idx_son_i = small.tile([P, NCH_SON], I32)
        idx8 = small.tile([P, NCH * 16 * 8], U32)  # raw max_index outputs

        near_in = dram.tile([SON_ROWS, 1], I32)
        near_all = dram.tile([NOBJ, 1], I32)

        # per-token-chunk state created during the pipelined loop
        pf_t = {}      # chunk -> gathered feats tile [P, 512]
        cj_t = {}      # chunk -> cond tile [P, 8, 128]
        pend = {}      # chunk -> argmin tile awaiting deferred gather

        def emit_gather(c, idxc):
            """Right after chunk c's argmin: compose with sonata + gather
            feats rows + kick off this token-chunk's cond DMA."""
            b, j = divmod(c - NCH_SON, 8)
            compj = compp.tile([P, 1], I32, name="compj", tag="compj")
            nc.gpsimd.indirect_dma_start(
                out=compj[:], out_offset=None, in_=near_all[:],
                in_offset=IndirectOffsetOnAxis(ap=idxc[:], axis=0))
            pfj = pfp.tile([P, DFEAT], BF16, name="pfj", tag="pfj")
            nc.gpsimd.indirect_dma_start(
                out=pfj[:], out_offset=None, in_=feats[:],
                in_offset=IndirectOffsetOnAxis(ap=compj[:], axis=0))
            pf_t[c] = pfj
            cj = condpool.tile([P, DLAT // P, P], BF16, name="cj", tag="cj")
            nc.sync.dma_start(cj[:], condP[b, :, j])
            cj_t[c] = cj

        def emit_proj(c):
            """Two chunks later: transpose gathered feats and project."""
            b, j = divmod(c - NCH_SON, 8)
            pfj = pf_t.pop(c)
            cj = cj_t.pop(c)
            ftj = ftp.tile([P, DFEAT // P, P], BF16, name="ftj", tag="ftj")
            for fc in range(DFEAT // P):
                pt = pstr.tile([P, P], BF16, name="pt", tag="pt")
                nc.tensor.transpose(pt[:], pfj[:, fc * P:(fc + 1) * P],
                                    ident_bf[:])
                nc.scalar.copy(ftj[:, fc, :], pt[:])
            out_dram = obj_out if b == 0 else geo_out
            for dh in range(DOUT // 512):
                pt = pspj.tile([P, 512], F32, name="ppj", tag="ppj")
                nc.tensor.matmul(
                    pt[:], ones_bf[:],
                    bias_sb[:, b * DOUT + dh * 512: b * DOUT + (dh + 1) * 512],
                    start=True, stop=False)
                for kc in range(DLAT // P):
                    nc.tensor.matmul(
                        pt[:], cj[:, kc, :],
                        wsb[b][:, kc, dh * 512:(dh + 1) * 512],
                        start=False, stop=False)
                nf = DFEAT // P
                for fc in range(nf):
                    nc.tensor.matmul(
                        pt[:], ftj[:, fc, :],
                        wsb[b][:, DLAT // P + fc, dh * 512:(dh + 1) * 512],
                        start=False, stop=(fc == nf - 1))
                ot = outp.tile([P, 512], F32, name="ot", tag="ot")
                nc.scalar.copy(ot[:], pt[:])
                nc.sync.dma_start(
                    out_dram[j * P:(j + 1) * P, dh * 512:(dh + 1) * 512],
                    ot[:])

        # ================= pipelined main loop =================
        # chunk list: (coords tile, query col offset)
        chunks = [(cs, M + j * P) for j in range(NCH_SON)] + \
                 [(cg, NOBJ + j * P) for j in range(NCH_GL)]

        def value_start(c):
            # transposed score stream -> Pool partition-reduce -> DRAM fold
            # -> V[q] broadcast tile.  V is bit-identical to the forward
            # scores (same K accumulation order), so max_index finds it.
            arbank = arp.tile([1, 16 * 512], F32, name="arbank", tag="ar")
            return arbank

        def value_tile(c, t, arbank):
            ct, a_off = chunks[c]
            pt = pst.tile([P, 512], F32, name="pv", tag="pv")
            for b in range(4):
                co = (t * 4 + b) * P
                nc.tensor.matmul(pt[:, b * P:(b + 1) * P],
                                 ct[:, co:co + P],
                                 ct[:, a_off:a_off + P],
                                 start=True, stop=True)
            stg = stgp.tile([P, 512], F32, name="stg", tag="stg")
            nc.scalar.copy(stg[:], pt[:])
            nc.gpsimd.tensor_reduce(arbank[:, t * 512:(t + 1) * 512],
                                    stg[:], axis=Axis.C, op=Alu.max)

        def value_finish(c, arbank):
            fold = dram.tile([1, 16 * 512], F32, name="fold", tag="fold")
            vrow = dram.tile([1, P], F32, name="vrow", tag="vrow")
            nc.sync.dma_start(fold[:], arbank[:])
            f2 = f2p.tile([64, P], F32, name="f2", tag="f2")
            nc.sync.dma_start(
                f2[:], fold[:].rearrange("o (t g q) -> (o t g) q", g=4, q=P))
            r2 = f2p.tile([1, P], F32, name="r2", tag="r2")
            nc.gpsimd.tensor_reduce(r2[:], f2[:], axis=Axis.C, op=Alu.max)
            nc.sync.dma_start(vrow[:], r2[:])
            vcol = vm8.tile([P, 1], F32, name="vcol", tag="vcol")
            nc.sync.dma_start(vcol[:], vrow[:].rearrange("o q -> q o"))
            vm = vm8.tile([P, 8], F32, name="vm", tag="vm")
            nc.vector.scalar_tensor_tensor(out=vm[:], in0=ones8[:],
                                           scalar=vcol[:], in1=ones8[:],
                                           op0=Alu.mult, op1=Alu.mult)
            return vm

        def emit_value(c):
            arbank = value_start(c)
            for t in range(16):
                value_tile(c, t, arbank)
            return value_finish(c, arbank)

        vms = {}
        for c, (ct, a_off) in enumerate(chunks):
            if c == 0:
                vms[0] = emit_value(0)
                vms[1] = emit_value(1)
            nxt = c + 2 if c + 2 < NCH else None
            ab = value_start(nxt) if nxt is not None else None
            vm = vms.pop(c)
            # forward stream (scans straight from PSUM) interleaved with the
            # next-next chunk's transposed value tiles so the in-order PE
            # queue always has ready work while pst/psc buffers drain
            for q in range(16):
                if nxt is not None:
                    value_tile(nxt, q, ab)
                ps = psc.tile([P, 512], F32, name="ps", tag="ps")
                nc.tensor.matmul(ps[:], ct[:, a_off:a_off + P],
                                 ct[:, q * 512:(q + 1) * 512],
                                 start=True, stop=True)
                o = (c * 16 + q) * 8
                nc.vector.max_index(idx8[:, o:o + 8], vm[:], ps[:])
            if nxt is not None:
                vms[nxt] = value_finish(nxt, ab)
            # combine stripes: global first index = min(q*512 + idx_q)
            # (not-found -> 2^32-1, exact in f32 ordering)
            iv = idx8[:, c * 128:(c + 1) * 128].rearrange(
                "p (q e) -> p q e", e=8)
            fq = vm8.tile([P, 16], F32, name="fq", tag="fq")
            nc.vector.tensor_copy(fq[:], iv[:, :, 0])
            nc.vector.tensor_tensor(out=fq[:], in0=fq[:], in1=qb16[:],
                                    op=Alu.add)
            if c < NCH_SON:
                nc.vector.tensor_reduce(fmin_son[:, c:c + 1], fq[:],
                                        axis=Axis.X, op=Alu.min)
                if c == NCH_SON - 1:
                    nc.vector.tensor_copy(idx_son_i[:], fmin_son[:])
                    nc.sync.dma_start(
                        near_in[:].rearrange("(j p) o -> p (j o)", p=P),
                        idx_son_i[:])
                    nc.gpsimd.collective_compute(
                        "AllGather", Alu.bypass,
                        replica_groups=[list(range(NCORES))],
                        ins=[near_in[:]], outs=[near_all[:]])
            else:
                idxf = idxp.tile([P, 1], F32, name="idxf", tag="idxf")
                nc.vector.tensor_reduce(idxf[:], fq[:], axis=Axis.X,
                                        op=Alu.min)
                idxc = idxp.tile([P, 1], I32, name="idxc", tag="idxc")
                nc.vector.tensor_copy(idxc[:], idxf[:])
                pend[c] = idxc
                if c - 1 in pend:
                    emit_gather(c - 1, pend.pop(c - 1))
                if c - 3 >= NCH_SON:
                    emit_proj(c - 3)

        emit_gather(NCH - 1, pend.pop(NCH - 1))
        emit_proj(NCH - 3)
        emit_proj(NCH - 2)
        emit_proj(NCH - 1)


_NC_CACHE = None


def _get_nc():
    global _NC_CACHE
    if _NC_CACHE is None:
        _NC_CACHE = build_nc()
    return _NC_CACHE


def _split3(x):
    """fp32 -> three bf16 planes whose sum carries ~24 mantissa bits."""
    import ml_dtypes
    BF = ml_dtypes.bfloat16
    x = np.asarray(x, np.float32)
    b0 = x.astype(BF)
    r = x - b0.astype(np.float32)
    b1 = r.astype(BF)
    r2 = r - b1.astype(np.float32)
    b2 = r2.astype(BF)
    return b0, b1, b2


def _pack_queries(q):
    # [n, 3] -> [32, n] rows: split3 of (2x, 2y, 2z, 1) ordered
    # [q0,q0,q1,q0,q1,q2,q1,q2] pairing cands [c0,c1,c0,c2,c1,c0,c2,c1]
    q = np.asarray(q, np.float32)
    aug = np.concatenate([2.0 * q.T, np.ones((1, q.shape[0]), np.float32)], axis=0)
    q0, q1, q2 = _split3(aug)
    return np.ascontiguousarray(
        np.concatenate([q0, q0, q1, q0, q1, q2, q1, q2], axis=0))


def _pack_cands(p):
    # [m, 3] -> [32, m] rows: split3 of (x, y, z, -||p||^2)
    p = np.asarray(p, np.float32)
    aug = np.concatenate([p.T, -np.sum(p * p, axis=1)[None, :]], axis=0)
    c0, c1, c2 = _split3(aug)
    return np.ascontiguousarray(
        np.concatenate([c0, c1, c0, c2, c1, c0, c2, c1], axis=0))


def make_in_maps(object_surface, precomputed_points, precomputed_feats,
                 geo_cond, obj_cond, local_pc_infos, global_pc_infos,
                 W_obj, b_obj, W_geo, b_geo):
    import ml_dtypes
    BF = ml_dtypes.bfloat16

    obj_pts = np.asarray(object_surface, np.float32)[..., :3]
    pts0 = obj_pts[0]
    bT_son = _pack_cands(np.asarray(precomputed_points, np.float32))
    aT_gl = np.ascontiguousarray(np.concatenate(
        [_pack_queries(np.asarray(global_pc_infos, np.float32)[0]),
         _pack_queries(np.asarray(local_pc_infos, np.float32)[0])], axis=1))

    # Wp[br, p, kc, d] = W[kc*128+p, d]
    Ws = np.stack([W_obj, W_geo]).astype(np.float32)
    Wp = np.ascontiguousarray(
        Ws.reshape(2, (DLAT + DFEAT) // P, P, DOUT).transpose(0, 2, 1, 3)
    ).astype(BF)
    bias2 = np.ascontiguousarray(np.stack([b_obj, b_geo]).astype(BF))
    feats = np.ascontiguousarray(np.asarray(precomputed_feats).astype(BF))

    in_maps = []
    for c in range(NCORES):
        # condP[br, p, jc, kc, t] = cond[br][jc*128+t, kc*128+p]
        conds = np.stack([np.asarray(obj_cond)[c], np.asarray(geo_cond)[c]])
        condP = np.ascontiguousarray(
            conds.reshape(2, NTOK // P, P, DLAT // P, P).transpose(0, 4, 1, 3, 2)
        ).astype(BF)
        in_maps.append({
            "aT_son": _pack_queries(pts0[c * SON_ROWS:(c + 1) * SON_ROWS]),
            "bT_son": bT_son,
            "aT_gl": aT_gl,
            "bT_gl": _pack_cands(obj_pts[c]),
            "condP": condP,
            "Wp": Wp,
            "bias2": bias2,
            "feats": feats,
        })
    return in_maps


def kernel(**inputs):
    nc = _get_nc()
    in_maps = make_in_maps(**inputs)
    res = run_bass_kernel_spmd(nc, in_maps, core_ids=list(range(NCORES)))
    obj = np.stack([np.asarray(res.results[c]["obj_out"]).astype(np.float32)
                    for c in range(NCORES)])
    geo = np.stack([np.asarray(res.results[c]["geo_out"]).astype(np.float32)
                    for c in range(NCORES)])
    return obj, geo


if __name__ == "__main__":
    nc = _get_nc()
    print("built + compiled ok")
